# revision 5
# baseline (speedup 1.0000x reference)
"""LightGCN encoder (3-hop SpMM + VQ) on 8 Trainium2 NeuronCores.

Strategy (1D graph partitioning, per sharding hint):
  - Nodes (rows of the concatenated embedding table / segment_sum output)
    are sharded across the 8 cores; the edge list is partitioned by
    destination row.
  - Each hop: every core bulk-gathers source embeddings for its edges from
    a replicated bf16 node table in DRAM (dma_gather, 128B rows on a 256B
    stride), forms per-128-edge one-hot matrices scaled by the edge values
    (single DVE tensor_scalar each), and segment-sums into its destination
    shard via bf16 matmuls accumulating in fp32 PSUM.  Shards are then
    AllGathered into the next hop's table.
  - dma_gather indices are int16, so the permuted node table is split in 4
    chunks of 2 shards (25088 rows) and each core's edges are grouped by
    (source chunk, destination block), padded to 128-edge blocks.
  - The small 500x64 codebook is replicated; the final VQ (distances,
    argmin, quantize, loss) runs on-device, 512 users + 512 items per core.

Host-side work is limited to sharding/permuting inputs and unsharding
outputs; all FLOPs of the model run on the NeuronCores.
"""

import sys

for _p in ("/opt/trn_rl_repo",):
    if _p not in sys.path:
        sys.path.insert(0, _p)

import numpy as np
import ml_dtypes

import concourse.bass as bass
import concourse.bacc as bacc
import concourse.tile as tile
import concourse.mybir as mybir
from concourse.bass_utils import run_bass_kernel_spmd

DT = mybir.dt
F32 = DT.float32
BF16 = DT.bfloat16
I16 = DT.int16
I32 = DT.int32

N_LAYERS = 3
C1 = 0.25
C2 = 0.25
COST = 0.25
GT = 64          # gather tile: columns (128-edge blocks) per dma_gather


class Cfg:
    def __init__(self, U=50000, I=50000, E=2000000, B=4096, CODES=500, D=64,
                 NC=8):
        self.U, self.I, self.E, self.B, self.CODES, self.D, self.NC = (
            U, I, E, B, CODES, D, NC)
        self.N = U + I
        assert self.N % NC == 0
        self.SHARD = self.N // NC          # real nodes per core
        self.BLKS = -(-self.SHARD // 128)  # dest blocks of 128 rows per core
        self.SROWS = self.BLKS * 128       # padded rows per core shard
        self.NTOT = NC * self.SROWS        # padded global table rows
        # int16 gather indices: chunks of 2 shards
        self.NCHUNK = 4
        assert NC % self.NCHUNK == 0
        self.CROWS = (NC // self.NCHUNK) * self.SROWS
        assert self.CROWS <= 32767
        assert B % NC == 0
        self.PCB = B // NC                 # batch items per core
        self.BT = -(-self.PCB // 128)      # batch tiles of 128 per core

    def perm(self, n):
        """node id -> row in the permuted DRAM table"""
        s = n // self.SHARD
        l = n % self.SHARD
        return s * self.SROWS + (l % 128) * self.BLKS + (l // 128)


FULL = Cfg()


def _prep(cfg, user_emb, item_emb, codebook, adj_vals, adj_rows, adj_cols,
          users, items):
    """Host-side sharding: returns (in_maps, meta) with meta the per-
    (chunk, dest-block) 128-edge block counts (shared by all cores)."""
    NC, D = cfg.NC, cfg.D
    ego = np.concatenate([user_emb, item_emb], axis=0).astype(np.float32)

    node_perm = cfg.perm(np.arange(cfg.N))
    # hop-0 table: bf16, rows padded to 128 wide (256B stride, 128B data)
    x0_bf = np.zeros((cfg.NTOT, 2 * D), dtype=ml_dtypes.bfloat16)
    x0_bf[node_perm, :D] = ego.astype(ml_dtypes.bfloat16)
    ego_pad = np.zeros((cfg.NTOT, D), dtype=np.float32)
    ego_pad[node_perm] = ego
    ego_shards = ego_pad.reshape(NC, 128, cfg.BLKS * D)

    # --- edges: per core, group by (source chunk, dest block) ---
    shard_of = adj_rows // cfg.SHARD
    pcols = node_perm[adj_cols]
    qs = pcols // cfg.CROWS                 # source chunk
    lidx = (pcols % cfg.CROWS).astype(np.int16)

    NG = cfg.NCHUNK * cfg.BLKS
    cores = []
    cnts_all = np.zeros((NC, NG), dtype=np.int64)
    for c in range(NC):
        m = shard_of == c
        r = adj_rows[m] - c * cfg.SHARD
        g = qs[m] * cfg.BLKS + (r // 128)   # group id (chunk-major)
        order = np.argsort(g, kind="stable")
        cores.append((g[order], (r % 128)[order], lidx[m][order],
                      adj_vals[m][order].astype(np.float32)))
        cnts_all[c] = np.bincount(g[order], minlength=NG)
    nbq = np.maximum(1, -(-cnts_all.max(axis=0) // 128))   # blocks per group
    col_base = np.zeros(NG + 1, dtype=np.int64)
    np.cumsum(nbq, out=col_base[1:])
    NBLK = int(col_base[-1])

    in_maps = []
    iota = np.tile(np.arange(128, dtype=np.float32), (128, 1)).astype(
        ml_dtypes.bfloat16)
    ident = np.eye(128, dtype=np.float32)
    cbT = codebook.T.astype(np.float32).copy()
    cc2h = np.tile((codebook.astype(np.float32) ** 2).sum(1) * 0.5, (128, 1))
    for c in range(NC):
        g, r128, li, va = cores[c]
        gidx = np.zeros((NBLK, 128), dtype=np.int16)
        rloc = np.zeros((128, NBLK), dtype=np.float32)
        vls = np.zeros((128, NBLK), dtype=np.float32)
        gstart = np.zeros(NG + 1, dtype=np.int64)
        np.cumsum(cnts_all[c], out=gstart[1:])
        j = np.arange(len(g)) - gstart[g]          # rank within group
        bcol = col_base[g] + j // 128
        p = j % 128
        gidx[bcol, p] = li
        rloc[p, bcol] = r128.astype(np.float32)
        vls[p, bcol] = va
        # wrap the per-column indices for dma_gather: slot j -> [j%16, j//16]
        wrap = gidx.reshape(NBLK, 8, 16).transpose(2, 0, 1).reshape(
            16, NBLK * 8)
        goffs = np.tile(wrap, (8, 1))              # replicate to 128 parts

        uoffs = np.zeros((128, cfg.BT), dtype=np.int32)
        itoffs = np.zeros((128, cfg.BT), dtype=np.int32)
        mask = np.zeros((128, cfg.BT), dtype=np.float32)
        s = np.arange(cfg.PCB)
        pp, jj = s // cfg.BT, s % cfg.BT
        uoffs[pp, jj] = node_perm[users[c * cfg.PCB:(c + 1) * cfg.PCB]]
        itoffs[pp, jj] = node_perm[cfg.U + items[c * cfg.PCB:(c + 1) * cfg.PCB]]
        mask[pp, jj] = 1.0

        in_maps.append({
            "x0": x0_bf,
            "ego_shard": np.ascontiguousarray(ego_shards[c]),
            "goffs": goffs,
            "rloc": rloc,
            "vls": vls,
            "iota": iota,
            "ident": ident,
            "uoffs": uoffs,
            "itoffs": itoffs,
            "mask": mask,
            "cbT": cbT,
            "cc2h": cc2h.astype(np.float32),
            "cb": codebook.astype(np.float32),
        })
    return in_maps, tuple(int(x) for x in nbq)


def _gather_raw(eng, out_ap, in_ap, idxs_ap, num_idxs, elem_size,
                stride_bytes_256):
    """dma_gather with elem_size_bytes below the 256B transpose quantum
    (verified on hw: 128B reads on a 256B row stride)."""
    _in_ap = eng.lower_ap_dma(in_ap, for_custom_bir_dma=True)
    _idxs_ap = eng.lower_ap(idxs_ap)
    _out_ap = eng.lower_ap(out_ap)
    return eng.add_instruction(
        mybir.InstDMAGatherAnt(
            name=eng.bass.get_next_instruction_name(),
            ins=[*_in_ap, _idxs_ap, eng.lower_val_access(eng.to_reg(num_idxs))],
            outs=[_out_ap],
            transpose=False,
            num_idxs=num_idxs,
            elem_size=elem_size,
            stride_bytes_256=stride_bytes_256,
            gen_mode=0,
            single_packet=False,
            queue_num=0,
            sbuf_tokens_per_rank=0,
            sbuf_free_dim_per_rank=0,
            sbuf_free_dim_pad_per_rank=0,
            sbuf_byte_offset=0,
        ))


def _build(cfg, nbq):
    """Builds the SPMD Bass program (identical for all cores)."""
    nbq = np.asarray(nbq, dtype=np.int64).reshape(cfg.NCHUNK, cfg.BLKS)
    col_base = np.zeros(cfg.NCHUNK * cfg.BLKS + 1, dtype=np.int64)
    np.cumsum(nbq.ravel(), out=col_base[1:])
    NBLK = int(col_base[-1])
    D, CODES, BT = cfg.D, cfg.CODES, cfg.BT

    nc = bacc.Bacc("TRN2", target_bir_lowering=False, debug=False,
                   num_devices=cfg.NC)

    x0_t = nc.dram_tensor("x0", [cfg.NTOT, 2 * D], BF16, kind="ExternalInput")
    egos_t = nc.dram_tensor("ego_shard", [128, cfg.BLKS * D], F32,
                            kind="ExternalInput")
    goffs_t = nc.dram_tensor("goffs", [128, NBLK * 8], I16,
                             kind="ExternalInput")
    rloc_t = nc.dram_tensor("rloc", [128, NBLK], F32, kind="ExternalInput")
    vls_t = nc.dram_tensor("vls", [128, NBLK], F32, kind="ExternalInput")
    iota_t = nc.dram_tensor("iota", [128, 128], BF16, kind="ExternalInput")
    ident_t = nc.dram_tensor("ident", [128, 128], F32, kind="ExternalInput")
    uoffs_t = nc.dram_tensor("uoffs", [128, BT], I32, kind="ExternalInput")
    itoffs_t = nc.dram_tensor("itoffs", [128, BT], I32, kind="ExternalInput")
    mask_t = nc.dram_tensor("mask", [128, BT], F32, kind="ExternalInput")
    cbT_t = nc.dram_tensor("cbT", [64, CODES], F32, kind="ExternalInput")
    cc2h_t = nc.dram_tensor("cc2h", [128, CODES], F32, kind="ExternalInput")
    cb_t = nc.dram_tensor("cb", [CODES, D], F32, kind="ExternalInput")

    uout_t = nc.dram_tensor("u_out", [128, BT * D], F32, kind="ExternalOutput")
    ioout_t = nc.dram_tensor("io_out", [128, BT * D], F32,
                             kind="ExternalOutput")
    loss_t = nc.dram_tensor("loss_out", [128, BT], F32, kind="ExternalOutput")

    rg = [list(range(cfg.NC))]

    # static schedule: per chunk, gather tiles of <=GT columns
    gathers = []        # (q, col0, ncols)
    for q in range(cfg.NCHUNK):
        q0 = int(col_base[q * cfg.BLKS])
        q1 = int(col_base[(q + 1) * cfg.BLKS])
        c0 = q0
        while c0 < q1:
            ncols = min(GT, q1 - c0)
            gathers.append((q, c0, ncols))
            c0 += ncols
    # column -> (chunk, dest block, pos in chain, chain len)
    colinfo = []
    for q in range(cfg.NCHUNK):
        for d in range(cfg.BLKS):
            n = int(nbq[q, d])
            for i in range(n):
                colinfo.append((q, d, i, n))

    with tile.TileContext(nc) as tc:
        with (
            tc.tile_pool(name="cpool", bufs=1) as cpool,
            tc.tile_pool(name="dpool", bufs=1, space="DRAM") as dpool,
        ):
            goffs = cpool.tile([128, NBLK * 8], I16, tag="goffs")
            rloc = cpool.tile([128, NBLK], F32, tag="rloc")
            vls = cpool.tile([128, NBLK], F32, tag="vls")
            iota = cpool.tile([128, 128], BF16, tag="iota")
            acc = cpool.tile([128, cfg.BLKS * D], F32, tag="acc")
            hx = cpool.tile([128, cfg.BLKS * D], F32, tag="hx")
            stg16 = cpool.tile([128, cfg.BLKS * 2 * D], BF16, tag="stg16")
            nc.sync.dma_start(out=goffs[:], in_=goffs_t[:, :])
            nc.sync.dma_start(out=rloc[:], in_=rloc_t[:, :])
            nc.sync.dma_start(out=vls[:], in_=vls_t[:, :])
            nc.sync.dma_start(out=iota[:], in_=iota_t[:, :])
            nc.sync.dma_start(out=acc[:], in_=egos_t[:, :])
            nc.vector.memset(stg16[:], 0)

            xshard = dpool.tile([cfg.SROWS, 2 * D], BF16, tag="xshard")
            xfulls = [dpool.tile([cfg.NTOT, 2 * D], BF16, tag=f"xfull{h}",
                                 name=f"xfull{h}", addr_space="Shared")
                      for h in range(N_LAYERS - 1)]
            accs = dpool.tile([cfg.SROWS, D], F32, tag="accs")
            accf = dpool.tile([cfg.NTOT, D], F32, tag="accf",
                              addr_space="Shared")

            with (
                tc.tile_pool(name="gpool", bufs=2) as gpool,
                tc.tile_pool(name="rpool", bufs=4) as rpool,
                tc.tile_pool(name="ppool", bufs=8, space="PSUM") as ppool,
            ):
                for hop in range(N_LAYERS):
                    src = x0_t if hop == 0 else xfulls[hop - 1]
                    ps = None
                    for (q, col0, ncols) in gathers:
                        gt = gpool.tile([128, GT * D], BF16, tag="G")
                        _gather_raw(
                            nc.gpsimd,
                            out_ap=gt[:, :ncols * D].rearrange(
                                "p (s f) -> p s f", s=ncols),
                            in_ap=src[q * cfg.CROWS:(q + 1) * cfg.CROWS, :],
                            idxs_ap=goffs[:, col0 * 8:(col0 + ncols) * 8],
                            num_idxs=ncols * 128,
                            elem_size=D,
                            stride_bytes_256=1,
                        )
                        for j in range(ncols):
                            b = col0 + j
                            _, d, i, n = colinfo[b]
                            if i == 0:
                                ps = ppool.tile([128, D], F32, tag="ps")
                            R = rpool.tile([128, 128], BF16, tag="R")
                            nc.vector.tensor_scalar(
                                out=R[:], in0=iota[:],
                                scalar1=rloc[:, b:b + 1],
                                scalar2=vls[:, b:b + 1],
                                op0=mybir.AluOpType.is_equal,
                                op1=mybir.AluOpType.mult)
                            nc.tensor.matmul(
                                out=ps[:], lhsT=R[:],
                                rhs=gt[:, j * D:(j + 1) * D],
                                start=(i == 0), stop=(i == n - 1))
                            if i == n - 1:
                                dsl = slice(d * D, (d + 1) * D)
                                if q == 0:
                                    nc.vector.tensor_copy(out=hx[:, dsl],
                                                          in_=ps[:])
                                else:
                                    nc.vector.tensor_add(
                                        out=hx[:, dsl], in0=hx[:, dsl],
                                        in1=ps[:])
                    nc.vector.tensor_add(out=acc[:], in0=acc[:], in1=hx[:])
                    if hop < N_LAYERS - 1:
                        nc.vector.tensor_copy(
                            out=stg16[:].rearrange("p (b w) -> p b w",
                                                   w=2 * D)[:, :, 0:D],
                            in_=hx[:].rearrange("p (b f) -> p b f", f=D))
                        nc.sync.dma_start(
                            out=xshard[:].rearrange("(p b) f -> p (b f)",
                                                    p=128),
                            in_=stg16[:])
                        nc.gpsimd.collective_compute(
                            "AllGather", mybir.AluOpType.bypass,
                            replica_groups=rg,
                            ins=[xshard[:].opt()],
                            outs=[xfulls[hop][:].opt()])

            # mean over hops, AllGather the final fp32 table
            nc.vector.tensor_scalar_mul(hx[:], acc[:], 1.0 / (N_LAYERS + 1))
            nc.sync.dma_start(
                out=accs[:].rearrange("(p b) f -> p (b f)", p=128),
                in_=hx[:])
            nc.gpsimd.collective_compute(
                "AllGather", mybir.AluOpType.bypass, replica_groups=rg,
                ins=[accs[:].opt()], outs=[accf[:].opt()])

            # ---- finale: batch gathers + VQ ----
            with (
                tc.tile_pool(name="fpool", bufs=1) as fpool,
                tc.tile_pool(name="fppool", bufs=4, space="PSUM") as fpp,
            ):
                uoffs = fpool.tile([128, BT], I32, tag="uoffs")
                itoffs = fpool.tile([128, BT], I32, tag="itoffs")
                maskb = fpool.tile([128, BT], F32, tag="maskb")
                ident = fpool.tile([128, 128], F32, tag="ident")
                cbT = fpool.tile([64, CODES], F32, tag="cbT")
                cc2h = fpool.tile([128, CODES], F32, tag="cc2h")
                nc.sync.dma_start(out=uoffs[:], in_=uoffs_t[:, :])
                nc.sync.dma_start(out=itoffs[:], in_=itoffs_t[:, :])
                nc.sync.dma_start(out=maskb[:], in_=mask_t[:, :])
                nc.sync.dma_start(out=ident[:], in_=ident_t[:, :])
                nc.sync.dma_start(out=cbT[:], in_=cbT_t[:, :])
                nc.sync.dma_start(out=cc2h[:], in_=cc2h_t[:, :])

                ut = fpool.tile([128, BT * D], F32, tag="ut")
                it = fpool.tile([128, BT * D], F32, tag="it")
                for t in range(BT):
                    nc.gpsimd.indirect_dma_start(
                        out=ut[:, t * D:(t + 1) * D], out_offset=None,
                        in_=accf[:, :],
                        in_offset=bass.IndirectOffsetOnAxis(
                            ap=uoffs[:, t:t + 1], axis=0))
                    nc.gpsimd.indirect_dma_start(
                        out=it[:, t * D:(t + 1) * D], out_offset=None,
                        in_=accf[:, :],
                        in_offset=bass.IndirectOffsetOnAxis(
                            ap=itoffs[:, t:t + 1], axis=0))
                nc.sync.dma_start(out=uout_t[:, :], in_=ut[:])

                io = fpool.tile([128, BT * D], F32, tag="io")
                lsum = fpool.tile([128, BT], F32, tag="lsum")
                for t in range(BT):
                    itt = it[:, t * D:(t + 1) * D]
                    tps = fpp.tile([D, 128], F32, tag="tps")
                    nc.tensor.transpose(out=tps[:], in_=itt, identity=ident[:])
                    itT = fpool.tile([D, 128], F32, tag="itT")
                    nc.vector.tensor_copy(out=itT[:], in_=tps[:])
                    scs = fpp.tile([128, CODES], F32, tag="scs")
                    nc.tensor.matmul(out=scs[:], lhsT=itT[:], rhs=cbT[:],
                                     start=True, stop=True)
                    m = fpool.tile([128, CODES], F32, tag="m")
                    nc.vector.tensor_tensor(out=m[:], in0=scs[:], in1=cc2h[:],
                                            op=mybir.AluOpType.subtract)
                    mx = fpool.tile([128, 8], F32, tag="mx")
                    midx = fpool.tile([128, 8], DT.uint32, tag="midx")
                    nc.vector.max(mx[:], m[:])
                    nc.vector.max_index(midx[:], mx[:], m[:])
                    q = fpool.tile([128, D], F32, tag="q")
                    nc.gpsimd.indirect_dma_start(
                        out=q[:], out_offset=None, in_=cb_t[:, :],
                        in_offset=bass.IndirectOffsetOnAxis(
                            ap=midx[:, 0:1], axis=0))
                    iot = io[:, t * D:(t + 1) * D]
                    nc.vector.tensor_scalar_mul(iot, q[:], C1)
                    nc.vector.tensor_add(out=iot, in0=iot, in1=itt)
                    df = fpool.tile([128, D], F32, tag="df")
                    nc.vector.tensor_tensor(out=df[:], in0=itt, in1=q[:],
                                            op=mybir.AluOpType.subtract)
                    nc.vector.tensor_mul(out=df[:], in0=df[:], in1=df[:])
                    nc.vector.tensor_reduce(
                        out=lsum[:, t:t + 1], in_=df[:],
                        axis=mybir.AxisListType.X, op=mybir.AluOpType.add)
                nc.vector.tensor_mul(out=lsum[:], in0=lsum[:], in1=maskb[:])
                nc.sync.dma_start(out=ioout_t[:, :], in_=io[:])
                nc.sync.dma_start(out=loss_t[:, :], in_=lsum[:])

    nc.compile()
    return nc


_BUILD_CACHE = {}


def _get_program(cfg, nbq):
    key = (cfg.N, cfg.E, cfg.B, nbq)
    if key not in _BUILD_CACHE:
        _BUILD_CACHE[key] = _build(cfg, nbq)
    return _BUILD_CACHE[key]


def _unshard(cfg, results):
    B, D, NC = cfg.B, cfg.D, cfg.NC
    u = np.zeros((B, D), dtype=np.float32)
    io = np.zeros((B, D), dtype=np.float32)
    tot = 0.0
    for c in range(NC):
        r = results[c]
        u[c * cfg.PCB:(c + 1) * cfg.PCB] = (
            r["u_out"].reshape(128, cfg.BT, D).reshape(128 * cfg.BT, D)
            [:cfg.PCB])
        io[c * cfg.PCB:(c + 1) * cfg.PCB] = (
            r["io_out"].reshape(128, cfg.BT, D).reshape(128 * cfg.BT, D)
            [:cfg.PCB])
        tot += float(r["loss_out"].astype(np.float64).sum())
    loss = np.float32(C2 * (1.0 + COST) * tot / (B * D))
    return u, io, loss


def run(cfg, trace=False, **inputs):
    inputs = {k: np.asarray(v) for k, v in inputs.items()}
    in_maps, nbq = _prep(cfg, **inputs)
    nc = _get_program(cfg, nbq)
    rr = run_bass_kernel_spmd(nc, in_maps, core_ids=list(range(cfg.NC)),
                              trace=trace)
    u, io, loss = _unshard(cfg, rr.results)
    return (u, io, loss), rr


def kernel(**inputs):
    out, _ = run(FULL, trace=False, **inputs)
    return out
